# revision 13
# baseline (speedup 1.0000x reference)
"""CT self-attention (causal + 2 future frames) for Trainium2, 8 NeuronCores.

Sharding: batch (4-way) x head-group (2-way): core c = 2*b + g handles batch b,
heads [8g, 8g+8). Each core computes its QKV projection slice, banded
attention for its 8 heads, and a partial output projection; the host sums the
two partial outputs per batch and adds the (host-folded) biases.

Matmul operands are bf16 (PE streams 1 col/cycle either way, but bf16 weight
loads are ~3x faster than the f32r self-loading path and DMA traffic halves);
accumulation stays fp32 in PSUM.  Attention is computed transposed
(S_T[k, q]) so no on-device transposes are needed anywhere:
  - scores: S_T = K_h^T-tile.T @ Q_h  (2 heads packed in the 128-row PE array
    via tile_position row tiling, head A rows 0-63, head B rows 64-127)
  - CT mask: extra accumulating matmul -1e9*I @ MQ[off] into the scores PSUM
  - softmax: exp on ScalarE with scale=1/8 and per-key padding bias; no max
    subtraction (|s|/8 <= ~6 for N(0,1) inputs, fp32 exp is safe); the
    denominator comes for free from a ones column appended to V (M=65 AV
    matmul, denominator lands on partition 64)
  - AV: attnT = V-tile.T @ E, accumulated over key tiles.  The AV pair for
    key tile kt is deferred TWO kt steps so the next scores tiles reach the
    scalar engine before the PE consumes exp(kt): ScalarE (the phase-B
    bottleneck at ~935ns/kt) stays saturated instead of idling ~45%/kt.
  - normalize: denominator rows gathered (SBUF->SBUF DMA on the gpsimd
    queue, so they don't queue behind weight/output DMAs) into one [8, 512]
    tile, one batched DVE reciprocal per query block, a [8,128] selector
    matmul broadcasts rows 2p/2p+1 across the pair's 128 partitions, then one
    [128,512] multiply per pair writes bf16 AT
  - output projection: attnT chunks as lhsT (2 matmuls per weight load),
    emitted per query block so it overlaps the next block's attention
"""
import math
from contextlib import ExitStack

import numpy as np

B, T, D, H = 4, 2048, 1024, 16
HD = D // H            # 64
L = 2                  # max_future_frames
NCORES = 8
HPG = 8                # heads per group/core
NPAIR = 4              # head pairs per core
FCH = 8                # feature chunks (D / 128)
TQ5 = 4                # 512-wide query tiles
NKT = 16               # 128-wide key tiles
NEG = -1.0e9

_BUILT = {}


def _build_nc():
    import concourse.tile as tile
    from concourse import bacc, mybir

    dt = mybir.dt
    f32, f32r, bf16 = dt.float32, dt.float32r, dt.bfloat16
    Exp = mybir.ActivationFunctionType.Exp
    MUL = mybir.AluOpType.mult
    ADD = mybir.AluOpType.add

    f16 = dt.float16

    nc = bacc.Bacc(None, target_bir_lowering=False)
    xT_d = nc.dram_tensor("xT", [FCH, 128, T], bf16, kind="ExternalInput")
    wqkvT_d = nc.dram_tensor("wqkvT", [FCH, 128, 3 * 512], bf16, kind="ExternalInput")
    woutT_d = nc.dram_tensor("woutT", [NPAIR, 128, D], bf16, kind="ExternalInput")
    bq_d = nc.dram_tensor("bq", [128, NPAIR], f32, kind="ExternalInput")
    bk_d = nc.dram_tensor("bk", [128, NPAIR], f32, kind="ExternalInput")
    kpb_d = nc.dram_tensor("kpb", [128, NKT], f32, kind="ExternalInput")
    mq_d = nc.dram_tensor("mq", [128, 5, 512], bf16, kind="ExternalInput")
    mk_d = nc.dram_tensor("mk", [128, 128], bf16, kind="ExternalInput")
    vones_d = nc.dram_tensor("vones", [128, NKT * HPG], bf16, kind="ExternalInput")
    selbc_d = nc.dram_tensor("selbc", [2, 128], f16, kind="ExternalInput")
    out_d = nc.dram_tensor("out_part", [T, D], f32, kind="ExternalOutput")

    with tile.TileContext(nc) as tc, \
         nc.allow_low_precision(reason="bf16 matmul operands, fp32 accum"), \
         ExitStack() as top:
        pers = top.enter_context(tc.tile_pool(name="pers", bufs=1))
        QT = pers.tile([128, NPAIR, T], bf16, name="QT")
        KT = pers.tile([128, NPAIR, T], bf16, name="KT")
        Vt = pers.tile([128, NKT, HPG, HD + 1], bf16, name="Vt")
        mq_sb = pers.tile([128, 5, 512], bf16, name="mq_sb")
        mk_sb = pers.tile([128, 128], bf16, name="mk_sb")
        kp_sb = pers.tile([128, NKT], f32, name="kp_sb")
        bq_sb = pers.tile([128, NPAIR], f32, name="bq_sb")
        bk_sb = pers.tile([128, NPAIR], f32, name="bk_sb")
        selbc_sb = pers.tile([2, 128], f16, name="selbc_sb")
        vones_sb = pers.tile([128, NKT * HPG], bf16, name="vones_sb")

        # ---- Phase A: QKV projection ----
        # x^T resident; Q/K weight tiles streamed and kept stationary across
        # the 4 query blocks.  The bulk x/w loads are issued FIRST so the
        # small preamble DMAs don't delay the first projection matmul.
        with tc.tile_pool(name="wq", bufs=12) as wqp, \
             tc.tile_pool(name="wv", bufs=1) as wvp, \
             tc.tile_pool(name="xs", bufs=1) as xsp, \
             tc.tile_pool(name="psA", bufs=1, space="PSUM") as psA, \
             tc.tile_pool(name="psV", bufs=2, space="PSUM") as psV:
            xT_sb = xsp.tile([128, FCH, T], bf16, name="xT_sb")
            for f in range(FCH):
                nc.gpsimd.dma_start(xT_sb[:, f, :], xT_d[f])
            nc.gpsimd.dma_start(bq_sb[:], bq_d[:])
            nc.gpsimd.dma_start(bk_sb[:], bk_d[:])
            nc.gpsimd.dma_start(kp_sb[:], kpb_d[:])
            nc.gpsimd.dma_start(mq_sb[:], mq_d[:])
            nc.gpsimd.dma_start(mk_sb[:], mk_d[:])
            nc.gpsimd.dma_start(selbc_sb[:], selbc_d[:])
            nc.gpsimd.dma_start(vones_sb[:], vones_d[:])
            wqv_sb = wvp.tile([128, FCH, 512], bf16, name="wqv_sb")
            for f in range(FCH):
                nc.gpsimd.dma_start(wqv_sb[:, f, :], wqkvT_d[f, :, 1024:1536])
            nc.vector.tensor_copy(
                Vt[:, :, :, HD],
                vones_sb[:].rearrange("p (a b) -> p a b", a=NKT))
            for tgt in range(8):
                pqks = [psA.tile([128, 512], f32, name=f"pqk{t5}", tag=f"pqk{t5}")
                        for t5 in range(TQ5)]
                for f in range(FCH):
                    wqt = wqp.tile([128, 128], bf16, name="wqt", tag="wqt")
                    nc.sync.dma_start(
                        wqt[:], wqkvT_d[f, :, tgt * 128:(tgt + 1) * 128])
                    for t5 in range(TQ5):
                        nc.tensor.matmul(
                            pqks[t5][:], wqt[:],
                            xT_sb[:, f, t5 * 512:(t5 + 1) * 512],
                            start=(f == 0), stop=(f == FCH - 1))
                pair = tgt % 4
                for t5 in range(TQ5):
                    dst = (QT if tgt < 4 else KT)[:, pair, t5 * 512:(t5 + 1) * 512]
                    bias = (bq_sb if tgt < 4 else bk_sb)[:, pair:pair + 1]
                    nc.vector.tensor_scalar(dst, pqks[t5][:], bias, None, ADD)
            # V in [t, ch] layout, 128-query subtiles
            for t in range(16):
                pv = psV.tile([128, 512], f32, name="pv", tag="pv0")
                for f in range(FCH):
                    nc.tensor.matmul(
                        pv[:], xT_sb[:, f, t * 128:(t + 1) * 128],
                        wqv_sb[:, f, :],
                        start=(f == 0), stop=(f == FCH - 1))
                nc.vector.tensor_copy(
                    Vt[:, t, :, 0:HD],
                    pv[:].rearrange("p (h d) -> p h d", h=HPG))

        # ---- Phases B+C: banded attention fused with output projection ----
        with tc.tile_pool(name="pers2", bufs=1) as pers2:
            AT = pers2.tile([128, NPAIR, T], bf16, name="AT")

            with tc.tile_pool(name="eps", bufs=4) as epool, \
                 tc.tile_pool(name="nsb", bufs=2) as nsb, \
                 tc.tile_pool(name="avp", bufs=1) as avp, \
                 tc.tile_pool(name="wo", bufs=1) as wop, \
                 tc.tile_pool(name="osb", bufs=2) as osb, \
                 tc.tile_pool(name="psAv", bufs=1, space="PSUM") as psAv, \
                 tc.tile_pool(name="psSc", bufs=2, space="PSUM") as psSc, \
                 tc.tile_pool(name="psC", bufs=1, space="PSUM") as psC:
                wo_sb = wop.tile([128, NPAIR, D], bf16, name="wo_sb")
                for cchunk in range(NPAIR):
                    nc.gpsimd.dma_start(wo_sb[:, cchunk, :], woutT_d[cchunk])

                def mk_normalize(q5, p, avsA, avsB, dpl):
                    # per-pair chain: fast reciprocal of the two denominator
                    # rows, f16 copy, selector matmul broadcasting row j//64
                    # to partition j, two multiplies writing the bf16 AT
                    # slice (head A partitions 0-63, head B 64-127).
                    def emit():
                        qs = slice(q5 * 512, (q5 + 1) * 512)
                        rf = nsb.tile([2, 512], f32, name="rf", tag="rf")
                        nc.vector.reciprocal_approx_fast(rf[:], dpl[:])
                        rh = nsb.tile([2, 512], f16, name="rh", tag="rh")
                        nc.vector.tensor_copy(rh[:], rf[:])
                        bc = psSc.tile([128, 512], f32, name="bc", tag="sc2")
                        nc.tensor.matmul(bc[:], selbc_sb[:], rh[:],
                                         start=True, stop=True)
                        nc.vector.tensor_tensor(
                            AT[0:64, p, qs], avsA[0:64, :], bc[0:64, :], MUL)
                        nc.vector.tensor_tensor(
                            AT[64:128, p, qs], avsB[0:64, :], bc[64:128, :], MUL)
                    return emit

                def mk_proj(q5):
                    # output projection for the 4 query tiles of block q5
                    def emit():
                        for tq in range(4):
                            t = 4 * q5 + tq
                            tsl = slice(t * 128, (t + 1) * 128)
                            po0 = psC.tile([128, 512], f32, name="po0", tag="po0")
                            po1 = psC.tile([128, 512], f32, name="po1", tag="po1")
                            for cchunk in range(NPAIR):
                                lhsT = AT[:, cchunk, tsl]
                                nc.tensor.matmul(
                                    po0[:], lhsT, wo_sb[:, cchunk, 0:512],
                                    start=(cchunk == 0), stop=(cchunk == 3))
                                nc.tensor.matmul(
                                    po1[:], lhsT, wo_sb[:, cchunk, 512:1024],
                                    start=(cchunk == 0), stop=(cchunk == 3))
                            ot = osb.tile([128, D], f32, name="ot", tag="ot")
                            nc.vector.tensor_copy(ot[:, 0:512], po0[:])
                            nc.vector.tensor_copy(ot[:, 512:1024], po1[:])
                            nc.sync.dma_start(out_d[tsl, :], ot[:])
                    return emit

                defq = []  # deferred normalize/proj closures, FIFO
                for q5 in range(TQ5):
                    nkt = min(4 * q5 + 5, NKT)
                    q5s = q5 * 512
                    for p in range(NPAIR):
                        avA = psAv.tile([HD + 1, 512], f32, name="avA", tag="avA")
                        avB = psAv.tile([HD + 1, 512], f32, name="avB", tag="avB")
                        # depth-2 AV deferral queue: av(kt) runs after
                        # scores(kt+2) is issued, so exp(kt+1)/exp(kt+2) never
                        # wait behind it on the scalar engine.
                        pend_av = []
                        for kt in range(nkt):
                            ks = slice(kt * 128, (kt + 1) * 128)
                            off = kt - 4 * q5
                            masked = off >= 0
                            # masked tiles only affect queries >= q0
                            q0 = max(0, 128 * off - L) if masked else 0
                            qs = slice(q5s + q0, q5s + 512)
                            sc2 = psSc.tile([128, 2, 512], f32, name="sc2", tag="sc2")
                            nc.tensor.matmul(sc2[:, 0, q0:512],
                                             KT[0:64, p, ks], QT[0:64, p, qs],
                                             start=True, stop=not masked,
                                             tile_position=(0, 0))
                            nc.tensor.matmul(sc2[:, 1, q0:512],
                                             KT[64:128, p, ks], QT[64:128, p, qs],
                                             start=True, stop=not masked,
                                             tile_position=(64, 0))
                            if masked:
                                m1 = min(512, 128 * off + 126)
                                nc.tensor.matmul(sc2[:, 0, q0:m1], mk_sb[:],
                                                 mq_sb[:, off, q0:m1],
                                                 start=False, stop=True,
                                                 skip_group_check=True)
                                nc.tensor.matmul(sc2[:, 1, q0:m1], mk_sb[:],
                                                 mq_sb[:, off, q0:m1],
                                                 start=False, stop=True,
                                                 skip_group_check=True)
                            if len(pend_av) == 2:
                                pend_av.pop(0)()
                            e2 = epool.tile([128, 2, 512], bf16, name="e2", tag="e2")
                            nc.scalar.activation(e2[:, :, q0:512], sc2[:, :, q0:512],
                                                 Exp, bias=kp_sb[:, kt:kt + 1],
                                                 scale=1.0 / math.sqrt(HD))

                            def mk_av(kt=kt, e2=e2, q0=q0, avA=avA, avB=avB,
                                      p=p, nkt=nkt):
                                nc.tensor.matmul(avA[0:65, q0:512],
                                                 Vt[:, kt, 2 * p, :],
                                                 e2[:, 0, q0:512],
                                                 start=(kt == 0), stop=(kt == nkt - 1),
                                                 skip_group_check=True)
                                nc.tensor.matmul(avB[0:65, q0:512],
                                                 Vt[:, kt, 2 * p + 1, :],
                                                 e2[:, 1, q0:512],
                                                 start=(kt == 0), stop=(kt == nkt - 1),
                                                 skip_group_check=True)
                            pend_av.append(mk_av)
                            if kt in (2, 4) and defq:
                                defq.pop(0)()
                        for fn in pend_av:
                            fn()
                        # copy AV out of PSUM (frees banks) + gather the
                        # denominator rows on the gpsimd DMA queue (so they
                        # don't wait behind weight/output DMAs)
                        avsA = avp.tile([HD + 1, 512], f32,
                                        name=f"avsA{p}", tag=f"avsA{p}")
                        avsB = avp.tile([HD + 1, 512], f32,
                                        name=f"avsB{p}", tag=f"avsB{p}")
                        nc.vector.tensor_copy(avsA[:], avA[:])
                        nc.vector.tensor_copy(avsB[:], avB[:])
                        dpl = nsb.tile([2, 512], f32, name=f"dp{p}", tag=f"dp{p}")
                        nc.gpsimd.dma_start(dpl[0:1, :], avsA[64:65, :])
                        nc.gpsimd.dma_start(dpl[1:2, :], avsB[64:65, :])
                        defq.append(mk_normalize(q5, p, avsA, avsB, dpl))
                    defq.append(mk_proj(q5))
                # flush the last block's normalize + projection
                for fn in defq:
                    fn()

    nc.finalize()
    return nc


def _host_inputs(x, key_padding_mask, w_qkv, b_qkv, w_out):
    """Per-core input dicts."""
    import ml_dtypes

    f32 = np.float32
    b16 = ml_dtypes.bfloat16
    # masks (shared across cores)
    j = np.arange(128)[:, None]
    q = np.arange(512)[None, :]
    mq = np.zeros((128, 5, 512), f32)
    for off in range(5):
        mq[:, off, :] = (128 * off + j > q + L).astype(f32)
    mq = mq.astype(b16)
    mk = (NEG * np.eye(128, dtype=f32)).astype(b16)
    vones = np.ones((128, NKT * HPG), b16)
    # selbc[r, j] = 1 where j // 64 == r: broadcasts denominator row r of a
    # pair to partitions 64r..64r+63
    selbc = np.zeros((2, 128), np.float16)
    selbc[0, 0:64] = 1.0
    selbc[1, 64:128] = 1.0

    in_maps = []
    for c in range(NCORES):
        b, g = divmod(c, 2)
        # channel rows for this group's Q/K (pairs of heads -> 128 rows each)
        qrows = np.concatenate(
            [w_qkv[64 * (8 * g + 2 * p):64 * (8 * g + 2 * p) + 128] for p in range(NPAIR)])
        krows = np.concatenate(
            [w_qkv[D + 64 * (8 * g + 2 * p):D + 64 * (8 * g + 2 * p) + 128] for p in range(NPAIR)])
        vrows = w_qkv[2 * D + 512 * g:2 * D + 512 * g + 512]
        w_all = np.concatenate([qrows, krows, vrows], 0)          # [1536, 1024]
        wqkvT = np.ascontiguousarray(w_all.T).reshape(FCH, 128, 3 * 512)
        bq = np.stack(
            [b_qkv[64 * (8 * g + 2 * p):64 * (8 * g + 2 * p) + 128] for p in range(NPAIR)], 1)
        bk = np.stack(
            [b_qkv[D + 64 * (8 * g + 2 * p):D + 64 * (8 * g + 2 * p) + 128] for p in range(NPAIR)], 1)
        xT = np.ascontiguousarray(x[b].T).reshape(FCH, 128, T)
        woutT = np.ascontiguousarray(w_out.T[512 * g:512 * g + 512]).reshape(NPAIR, 128, D)
        kpb = np.ascontiguousarray(
            (NEG * key_padding_mask[b].astype(f32)).reshape(NKT, 128).T)
        in_maps.append({
            "xT": xT.astype(b16), "wqkvT": wqkvT.astype(b16),
            "woutT": woutT.astype(b16),
            "bq": bq.astype(f32), "bk": bk.astype(f32), "kpb": kpb.astype(f32),
            "mq": mq, "mk": mk, "vones": vones, "selbc": selbc,
        })
    return in_maps


def kernel(x, key_padding_mask, w_qkv, b_qkv, w_out, b_out):
    from concourse.bass_utils import run_bass_kernel_spmd

    x = np.asarray(x, np.float32)
    key_padding_mask = np.asarray(key_padding_mask)
    w_qkv = np.asarray(w_qkv, np.float32)
    b_qkv = np.asarray(b_qkv, np.float32)
    w_out = np.asarray(w_out, np.float32)
    b_out = np.asarray(b_out, np.float32)

    if "nc" not in _BUILT:
        _BUILT["nc"] = _build_nc()
    nc = _BUILT["nc"]

    in_maps = _host_inputs(x, key_padding_mask, w_qkv, b_qkv, w_out)
    res = run_bass_kernel_spmd(nc, in_maps, core_ids=list(range(NCORES)))
    out = np.empty((B, T, D), np.float32)
    for b in range(B):
        out[b] = res.results[2 * b]["out_part"] + res.results[2 * b + 1]["out_part"]
    # host-folded biases: b_out plus the V-bias pushed through the projection
    bv = b_qkv[2 * D:3 * D]
    out += (b_out + bv @ w_out.T)[None, None, :].astype(np.float32)
    return out


# revision 18
# speedup vs baseline: 1.0970x; 1.0970x over previous
"""CT self-attention (causal + 2 future frames) for Trainium2, 8 NeuronCores.

Sharding: batch (4-way) x head-group (2-way): core c = 2*b + g handles batch b,
heads [8g, 8g+8). Each core computes its QKV projection slice, banded
attention for its 8 heads, and a partial output projection; the host sums the
two partial outputs per batch and adds the (host-folded) biases.

Matmul operands are bf16 (PE streams 1 col/cycle either way, but bf16 weight
loads are ~3x faster than the f32r self-loading path and DMA traffic halves);
accumulation stays fp32 in PSUM.  Attention is computed transposed
(S_T[k, q]) so no on-device transposes are needed anywhere:
  - scores: S_T = K_h^T-tile.T @ Q_h  (2 heads packed in the 128-row PE array
    via tile_position row tiling, head A rows 0-63, head B rows 64-127)
  - CT mask: extra accumulating matmul -1e9*I @ MQ[off] into the scores PSUM
  - softmax: exp on ScalarE with scale=1/8 and per-key padding bias; no max
    subtraction (|s|/8 <= ~6 for N(0,1) inputs, fp32 exp is safe); the
    denominator comes for free from a ones column appended to V (M=65 AV
    matmul, denominator lands on partition 64)
  - AV: attnT = V-tile.T @ E, accumulated over key tiles.  The AV pair for
    key tile kt is deferred TWO kt steps so the next scores tiles reach the
    scalar engine before the PE consumes exp(kt): ScalarE (the phase-B
    bottleneck at ~935ns/kt) stays saturated instead of idling ~45%/kt.
  - normalize: denominator rows gathered (SBUF->SBUF DMA on the gpsimd
    queue, so they don't queue behind weight/output DMAs) into one [8, 512]
    tile, one batched DVE reciprocal per query block, a [8,128] selector
    matmul broadcasts rows 2p/2p+1 across the pair's 128 partitions, then one
    [128,512] multiply per pair writes bf16 AT
  - output projection: attnT chunks as lhsT (2 matmuls per weight load),
    emitted per query block so it overlaps the next block's attention
"""
import math
from contextlib import ExitStack

import numpy as np

B, T, D, H = 4, 2048, 1024, 16
HD = D // H            # 64
L = 2                  # max_future_frames
NCORES = 8
HPG = 8                # heads per group/core
NPAIR = 4              # head pairs per core
FCH = 8                # feature chunks (D / 128)
TQ5 = 4                # 512-wide query tiles
NKT = 16               # 128-wide key tiles
NEG = -1.0e9

_BUILT = {}


def _build_nc():
    import concourse.tile as tile
    from concourse import bacc, mybir

    dt = mybir.dt
    f32, f32r, bf16 = dt.float32, dt.float32r, dt.bfloat16
    Exp = mybir.ActivationFunctionType.Exp
    MUL = mybir.AluOpType.mult
    ADD = mybir.AluOpType.add

    nc = bacc.Bacc(None, target_bir_lowering=False)
    xT_d = nc.dram_tensor("xT", [FCH, 128, T], bf16, kind="ExternalInput")
    wqkvT_d = nc.dram_tensor("wqkvT", [FCH, 128, 3 * 512], bf16, kind="ExternalInput")
    woutT_d = nc.dram_tensor("woutT", [NPAIR, 128, D], bf16, kind="ExternalInput")
    bq_d = nc.dram_tensor("bq", [128, NPAIR], f32, kind="ExternalInput")
    bk_d = nc.dram_tensor("bk", [128, NPAIR], f32, kind="ExternalInput")
    kpb_d = nc.dram_tensor("kpb", [128, NKT], f32, kind="ExternalInput")
    mq_d = nc.dram_tensor("mq", [128, 5, 512], bf16, kind="ExternalInput")
    mk_d = nc.dram_tensor("mk", [128, 128], bf16, kind="ExternalInput")
    vones_d = nc.dram_tensor("vones", [128, NKT * HPG], bf16, kind="ExternalInput")
    out_d = nc.dram_tensor("out_part", [T, D], f32, kind="ExternalOutput")

    with tile.TileContext(nc) as tc, \
         nc.allow_low_precision(reason="bf16 matmul operands, fp32 accum"), \
         ExitStack() as top:
        pers = top.enter_context(tc.tile_pool(name="pers", bufs=1))
        QT = pers.tile([128, NPAIR, T], bf16, name="QT")
        KT = pers.tile([128, NPAIR, T], bf16, name="KT")
        Vt = pers.tile([128, NKT, HPG, HD + 1], bf16, name="Vt")
        mq_sb = pers.tile([128, 5, 512], bf16, name="mq_sb")
        mk_sb = pers.tile([128, 128], bf16, name="mk_sb")
        kp_sb = pers.tile([128, NKT], f32, name="kp_sb")
        bq_sb = pers.tile([128, NPAIR], f32, name="bq_sb")
        bk_sb = pers.tile([128, NPAIR], f32, name="bk_sb")
        vones_sb = pers.tile([128, NKT * HPG], bf16, name="vones_sb")

        # ---- Phase A: QKV projection ----
        # x^T resident; Q/K weight tiles streamed and kept stationary across
        # the 4 query blocks.  The bulk x/w loads are issued FIRST so the
        # small preamble DMAs don't delay the first projection matmul.
        with tc.tile_pool(name="wq", bufs=12) as wqp, \
             tc.tile_pool(name="wv", bufs=1) as wvp, \
             tc.tile_pool(name="xs", bufs=1) as xsp, \
             tc.tile_pool(name="psA", bufs=1, space="PSUM") as psA, \
             tc.tile_pool(name="psV", bufs=2, space="PSUM") as psV:
            xT_sb = xsp.tile([128, FCH, T], bf16, name="xT_sb")
            for f in range(FCH):
                nc.gpsimd.dma_start(xT_sb[:, f, :], xT_d[f])
            nc.gpsimd.dma_start(bq_sb[:], bq_d[:])
            nc.gpsimd.dma_start(bk_sb[:], bk_d[:])
            nc.gpsimd.dma_start(kp_sb[:], kpb_d[:])
            nc.gpsimd.dma_start(mq_sb[:], mq_d[:])
            nc.gpsimd.dma_start(mk_sb[:], mk_d[:])
            nc.gpsimd.dma_start(vones_sb[:], vones_d[:])
            wqv_sb = wvp.tile([128, FCH, 512], bf16, name="wqv_sb")
            for f in range(FCH):
                nc.gpsimd.dma_start(wqv_sb[:, f, :], wqkvT_d[f, :, 1024:1536])
            nc.vector.tensor_copy(
                Vt[:, :, :, HD],
                vones_sb[:].rearrange("p (a b) -> p a b", a=NKT))
            for tgt in range(8):
                pqks = [psA.tile([128, 512], f32, name=f"pqk{t5}", tag=f"pqk{t5}")
                        for t5 in range(TQ5)]
                for f in range(FCH):
                    wqt = wqp.tile([128, 128], bf16, name="wqt", tag="wqt")
                    nc.sync.dma_start(
                        wqt[:], wqkvT_d[f, :, tgt * 128:(tgt + 1) * 128])
                    for t5 in range(TQ5):
                        nc.tensor.matmul(
                            pqks[t5][:], wqt[:],
                            xT_sb[:, f, t5 * 512:(t5 + 1) * 512],
                            start=(f == 0), stop=(f == FCH - 1))
                pair = tgt % 4
                for t5 in range(TQ5):
                    dst = (QT if tgt < 4 else KT)[:, pair, t5 * 512:(t5 + 1) * 512]
                    bias = (bq_sb if tgt < 4 else bk_sb)[:, pair:pair + 1]
                    nc.vector.tensor_scalar(dst, pqks[t5][:], bias, None, ADD)
            # V in [t, ch] layout, 128-query subtiles
            for t in range(16):
                pv = psV.tile([128, 512], f32, name="pv", tag="pv0")
                for f in range(FCH):
                    nc.tensor.matmul(
                        pv[:], xT_sb[:, f, t * 128:(t + 1) * 128],
                        wqv_sb[:, f, :],
                        start=(f == 0), stop=(f == FCH - 1))
                nc.vector.tensor_copy(
                    Vt[:, t, :, 0:HD],
                    pv[:].rearrange("p (h d) -> p h d", h=HPG))

        # ---- Phases B+C: banded attention fused with output projection ----
        with tc.tile_pool(name="pers2", bufs=1) as pers2:
            AT = pers2.tile([128, NPAIR, T], bf16, name="AT")

            with tc.tile_pool(name="eps", bufs=4) as epool, \
                 tc.tile_pool(name="nsb", bufs=2) as nsb, \
                 tc.tile_pool(name="avp", bufs=1) as avp, \
                 tc.tile_pool(name="wo", bufs=1) as wop, \
                 tc.tile_pool(name="osb", bufs=2) as osb, \
                 tc.tile_pool(name="psAv", bufs=1, space="PSUM") as psAv, \
                 tc.tile_pool(name="psSc", bufs=2, space="PSUM") as psSc, \
                 tc.tile_pool(name="psC", bufs=1, space="PSUM") as psC:
                wo_sb = wop.tile([128, NPAIR, D], bf16, name="wo_sb")
                for cchunk in range(NPAIR):
                    nc.gpsimd.dma_start(wo_sb[:, cchunk, :], woutT_d[cchunk])

                def mk_normalize(q5, p, avsA, avsB, dpl):
                    # per-pair chain, entirely off the tensor engine: fast
                    # reciprocal of the two gathered denominator rows (both
                    # on partition 0), gpsimd broadcast of that [1, 2, 512]
                    # row to all 128 partitions, then two multiplies writing
                    # the bf16 AT slice (head A partitions 0-63, head B
                    # 64-127, each reading its half of the broadcast).
                    def emit():
                        qs = slice(q5 * 512, (q5 + 1) * 512)
                        rf = nsb.tile([1, 2, 512], f32, name="rf", tag="rf")
                        nc.vector.reciprocal_approx_fast(rf[:], dpl[:])
                        bc2 = nsb.tile([64, 2, 512], f32, name="bc2", tag="bc2")
                        nc.gpsimd.partition_broadcast(bc2[:], rf[:])
                        nc.vector.tensor_tensor(
                            AT[0:64, p, qs], avsA[0:64, :], bc2[0:64, 0, :], MUL)
                        nc.vector.tensor_tensor(
                            AT[64:128, p, qs], avsB[0:64, :], bc2[0:64, 1, :], MUL)
                    return emit

                def mk_proj(q5):
                    # output projection for the 4 query tiles of block q5
                    def emit():
                        for tq in range(4):
                            t = 4 * q5 + tq
                            tsl = slice(t * 128, (t + 1) * 128)
                            po0 = psC.tile([128, 512], f32, name="po0", tag="po0")
                            po1 = psC.tile([128, 512], f32, name="po1", tag="po1")
                            for cchunk in range(NPAIR):
                                lhsT = AT[:, cchunk, tsl]
                                nc.tensor.matmul(
                                    po0[:], lhsT, wo_sb[:, cchunk, 0:512],
                                    start=(cchunk == 0), stop=(cchunk == 3))
                                nc.tensor.matmul(
                                    po1[:], lhsT, wo_sb[:, cchunk, 512:1024],
                                    start=(cchunk == 0), stop=(cchunk == 3))
                            ot = osb.tile([128, D], f32, name="ot", tag="ot")
                            nc.vector.tensor_copy(ot[:, 0:512], po0[:])
                            nc.vector.tensor_copy(ot[:, 512:1024], po1[:])
                            nc.sync.dma_start(out_d[tsl, :], ot[:])
                    return emit

                defq = []  # deferred normalize/proj closures, FIFO
                for q5 in range(TQ5):
                    nkt = min(4 * q5 + 5, NKT)
                    q5s = q5 * 512
                    for p in range(NPAIR):
                        avA = psAv.tile([HD + 1, 512], f32, name="avA", tag="avA")
                        avB = psAv.tile([HD + 1, 512], f32, name="avB", tag="avB")
                        # depth-2 AV deferral queue: av(kt) runs after
                        # scores(kt+2) is issued, so exp(kt+1)/exp(kt+2) never
                        # wait behind it on the scalar engine.
                        pend_av = []
                        for kt in range(nkt):
                            ks = slice(kt * 128, (kt + 1) * 128)
                            off = kt - 4 * q5
                            masked = off >= 0
                            # masked tiles only affect queries >= q0
                            q0 = max(0, 128 * off - L) if masked else 0
                            qs = slice(q5s + q0, q5s + 512)
                            sc2 = psSc.tile([128, 2, 512], f32, name="sc2", tag="sc2")
                            nc.tensor.matmul(sc2[:, 0, q0:512],
                                             KT[0:64, p, ks], QT[0:64, p, qs],
                                             start=True, stop=not masked,
                                             tile_position=(0, 0))
                            nc.tensor.matmul(sc2[:, 1, q0:512],
                                             KT[64:128, p, ks], QT[64:128, p, qs],
                                             start=True, stop=not masked,
                                             tile_position=(64, 0))
                            if masked:
                                m1 = min(512, 128 * off + 126)
                                nc.tensor.matmul(sc2[:, 0, q0:m1], mk_sb[:],
                                                 mq_sb[:, off, q0:m1],
                                                 start=False, stop=True,
                                                 skip_group_check=True)
                                nc.tensor.matmul(sc2[:, 1, q0:m1], mk_sb[:],
                                                 mq_sb[:, off, q0:m1],
                                                 start=False, stop=True,
                                                 skip_group_check=True)
                            if len(pend_av) == 2:
                                pend_av.pop(0)()
                            e2 = epool.tile([128, 2, 512], bf16, name="e2", tag="e2")
                            nc.scalar.activation(e2[:, :, q0:512], sc2[:, :, q0:512],
                                                 Exp, bias=kp_sb[:, kt:kt + 1],
                                                 scale=1.0 / math.sqrt(HD))

                            def mk_av(kt=kt, e2=e2, q0=q0, avA=avA, avB=avB,
                                      p=p, nkt=nkt):
                                nc.tensor.matmul(avA[0:65, q0:512],
                                                 Vt[:, kt, 2 * p, :],
                                                 e2[:, 0, q0:512],
                                                 start=(kt == 0), stop=(kt == nkt - 1),
                                                 skip_group_check=True)
                                nc.tensor.matmul(avB[0:65, q0:512],
                                                 Vt[:, kt, 2 * p + 1, :],
                                                 e2[:, 1, q0:512],
                                                 start=(kt == 0), stop=(kt == nkt - 1),
                                                 skip_group_check=True)
                            pend_av.append(mk_av)
                            if kt in (2, 4) and defq:
                                defq.pop(0)()
                        for fn in pend_av:
                            fn()
                        # copy AV out of PSUM (frees banks) + gather the
                        # denominator rows on the gpsimd DMA queue (so they
                        # don't wait behind weight/output DMAs)
                        avsA = avp.tile([HD + 1, 512], f32,
                                        name=f"avsA{p}", tag=f"avsA{p}")
                        avsB = avp.tile([HD + 1, 512], f32,
                                        name=f"avsB{p}", tag=f"avsB{p}")
                        nc.vector.tensor_copy(avsA[:], avA[:])
                        nc.vector.tensor_copy(avsB[:], avB[:])
                        dpl = nsb.tile([1, 2, 512], f32, name=f"dp{p}", tag=f"dp{p}")
                        nc.gpsimd.dma_start(dpl[0:1, 0, :], avsA[64:65, :])
                        nc.gpsimd.dma_start(dpl[0:1, 1, :], avsB[64:65, :])
                        defq.append(mk_normalize(q5, p, avsA, avsB, dpl))
                    defq.append(mk_proj(q5))
                # flush the last block's normalize + projection
                for fn in defq:
                    fn()

    nc.finalize()
    return nc


def _host_inputs(x, key_padding_mask, w_qkv, b_qkv, w_out):
    """Per-core input dicts."""
    import ml_dtypes

    f32 = np.float32
    b16 = ml_dtypes.bfloat16
    # masks (shared across cores)
    j = np.arange(128)[:, None]
    q = np.arange(512)[None, :]
    mq = np.zeros((128, 5, 512), f32)
    for off in range(5):
        mq[:, off, :] = (128 * off + j > q + L).astype(f32)
    mq = mq.astype(b16)
    mk = (NEG * np.eye(128, dtype=f32)).astype(b16)
    vones = np.ones((128, NKT * HPG), b16)

    in_maps = []
    for c in range(NCORES):
        b, g = divmod(c, 2)
        # channel rows for this group's Q/K (pairs of heads -> 128 rows each)
        qrows = np.concatenate(
            [w_qkv[64 * (8 * g + 2 * p):64 * (8 * g + 2 * p) + 128] for p in range(NPAIR)])
        krows = np.concatenate(
            [w_qkv[D + 64 * (8 * g + 2 * p):D + 64 * (8 * g + 2 * p) + 128] for p in range(NPAIR)])
        vrows = w_qkv[2 * D + 512 * g:2 * D + 512 * g + 512]
        w_all = np.concatenate([qrows, krows, vrows], 0)          # [1536, 1024]
        wqkvT = np.ascontiguousarray(w_all.T).reshape(FCH, 128, 3 * 512)
        bq = np.stack(
            [b_qkv[64 * (8 * g + 2 * p):64 * (8 * g + 2 * p) + 128] for p in range(NPAIR)], 1)
        bk = np.stack(
            [b_qkv[D + 64 * (8 * g + 2 * p):D + 64 * (8 * g + 2 * p) + 128] for p in range(NPAIR)], 1)
        xT = np.ascontiguousarray(x[b].T).reshape(FCH, 128, T)
        woutT = np.ascontiguousarray(w_out.T[512 * g:512 * g + 512]).reshape(NPAIR, 128, D)
        kpb = np.ascontiguousarray(
            (NEG * key_padding_mask[b].astype(f32)).reshape(NKT, 128).T)
        in_maps.append({
            "xT": xT.astype(b16), "wqkvT": wqkvT.astype(b16),
            "woutT": woutT.astype(b16),
            "bq": bq.astype(f32), "bk": bk.astype(f32), "kpb": kpb.astype(f32),
            "mq": mq, "mk": mk, "vones": vones,
        })
    return in_maps


def kernel(x, key_padding_mask, w_qkv, b_qkv, w_out, b_out):
    from concourse.bass_utils import run_bass_kernel_spmd

    x = np.asarray(x, np.float32)
    key_padding_mask = np.asarray(key_padding_mask)
    w_qkv = np.asarray(w_qkv, np.float32)
    b_qkv = np.asarray(b_qkv, np.float32)
    w_out = np.asarray(w_out, np.float32)
    b_out = np.asarray(b_out, np.float32)

    if "nc" not in _BUILT:
        _BUILT["nc"] = _build_nc()
    nc = _BUILT["nc"]

    in_maps = _host_inputs(x, key_padding_mask, w_qkv, b_qkv, w_out)
    res = run_bass_kernel_spmd(nc, in_maps, core_ids=list(range(NCORES)))
    out = np.empty((B, T, D), np.float32)
    for b in range(B):
        out[b] = res.results[2 * b]["out_part"] + res.results[2 * b + 1]["out_part"]
    # host-folded biases: b_out plus the V-bias pushed through the projection
    bv = b_qkv[2 * D:3 * D]
    out += (b_out + bv @ w_out.T)[None, None, :].astype(np.float32)
    return out


# revision 21
# speedup vs baseline: 1.1328x; 1.0326x over previous
"""CT self-attention (causal + 2 future frames) for Trainium2, 8 NeuronCores.

Sharding: batch (4-way) x head-group (2-way): core c = 2*b + g handles batch b,
heads [8g, 8g+8). Each core computes its QKV projection slice, banded
attention for its 8 heads, and a partial output projection; the host sums the
two partial outputs per batch and adds the (host-folded) biases.

Matmul operands are bf16 (PE streams 1 col/cycle either way, but bf16 weight
loads are ~3x faster than the f32r self-loading path and DMA traffic halves);
accumulation stays fp32 in PSUM.  Attention is computed transposed
(S_T[k, q]) so no on-device transposes are needed anywhere:
  - scores: S_T = K_h^T-tile.T @ Q_h  (2 heads packed in the 128-row PE array
    via tile_position row tiling, head A rows 0-63, head B rows 64-127)
  - CT mask: extra accumulating matmul -1e9*I @ MQ[off] into the scores PSUM
  - softmax: exp on ScalarE with scale=1/8 and per-key padding bias; no max
    subtraction (|s|/8 <= ~6 for N(0,1) inputs, fp32 exp is safe); the
    denominator comes for free from a ones column appended to V (M=65 AV
    matmul, denominator lands on partition 64)
  - AV: attnT = V-tile.T @ E, accumulated over key tiles.  The AV pair for
    key tile kt is deferred TWO kt steps so the next scores tiles reach the
    scalar engine before the PE consumes exp(kt): ScalarE (the phase-B
    bottleneck at ~935ns/kt) stays saturated instead of idling ~45%/kt.
  - normalize: denominator rows gathered (SBUF->SBUF DMA on the gpsimd
    queue, so they don't queue behind weight/output DMAs) into one [8, 512]
    tile, one batched DVE reciprocal per query block, a [8,128] selector
    matmul broadcasts rows 2p/2p+1 across the pair's 128 partitions, then one
    [128,512] multiply per pair writes bf16 AT
  - output projection: attnT chunks as lhsT (2 matmuls per weight load),
    emitted per query block so it overlaps the next block's attention
"""
import math
from contextlib import ExitStack

import numpy as np

B, T, D, H = 4, 2048, 1024, 16
HD = D // H            # 64
L = 2                  # max_future_frames
NCORES = 8
HPG = 8                # heads per group/core
NPAIR = 4              # head pairs per core
FCH = 8                # feature chunks (D / 128)
TQ5 = 4                # 512-wide query tiles
NKT = 16               # 128-wide key tiles
NEG = -1.0e9

_BUILT = {}


def _build_nc():
    import concourse.tile as tile
    from concourse import bacc, mybir

    dt = mybir.dt
    f32, f32r, bf16 = dt.float32, dt.float32r, dt.bfloat16
    Exp = mybir.ActivationFunctionType.Exp
    MUL = mybir.AluOpType.mult
    ADD = mybir.AluOpType.add

    nc = bacc.Bacc(None, target_bir_lowering=False)
    xT_d = nc.dram_tensor("xT", [FCH, 128, T], bf16, kind="ExternalInput")
    wqkvT_d = nc.dram_tensor("wqkvT", [FCH, 128, 3 * 512], bf16, kind="ExternalInput")
    woutT_d = nc.dram_tensor("woutT", [NPAIR, 128, D], bf16, kind="ExternalInput")
    bq_d = nc.dram_tensor("bq", [128, NPAIR], f32, kind="ExternalInput")
    bk_d = nc.dram_tensor("bk", [128, NPAIR], f32, kind="ExternalInput")
    kpb_d = nc.dram_tensor("kpb", [128, NKT], f32, kind="ExternalInput")
    mq_d = nc.dram_tensor("mq", [128, 5, 512], bf16, kind="ExternalInput")
    mk_d = nc.dram_tensor("mk", [128, 128], bf16, kind="ExternalInput")
    vones_d = nc.dram_tensor("vones", [128, NKT * HPG], bf16, kind="ExternalInput")
    out_d = nc.dram_tensor("out_part", [T, D], f32, kind="ExternalOutput")

    with tile.TileContext(nc) as tc, \
         nc.allow_low_precision(reason="bf16 matmul operands, fp32 accum"), \
         ExitStack() as top:
        pers = top.enter_context(tc.tile_pool(name="pers", bufs=1))
        QT = pers.tile([128, NPAIR, T], bf16, name="QT")
        KT = pers.tile([128, NPAIR, T], bf16, name="KT")
        Vt = pers.tile([128, NKT, HPG, HD + 1], bf16, name="Vt")
        mq_sb = pers.tile([128, 5, 512], bf16, name="mq_sb")
        mk_sb = pers.tile([128, 128], bf16, name="mk_sb")
        kp_sb = pers.tile([128, NKT], f32, name="kp_sb")
        bq_sb = pers.tile([128, NPAIR], f32, name="bq_sb")
        bk_sb = pers.tile([128, NPAIR], f32, name="bk_sb")
        vones_sb = pers.tile([128, NKT * HPG], bf16, name="vones_sb")

        # ---- Phase A: QKV projection ----
        # x^T resident; Q/K weight tiles streamed and kept stationary across
        # the 4 query blocks.  The bulk x/w loads are issued FIRST so the
        # small preamble DMAs don't delay the first projection matmul.
        with tc.tile_pool(name="wq", bufs=12) as wqp, \
             tc.tile_pool(name="wv", bufs=1) as wvp, \
             tc.tile_pool(name="xs", bufs=1) as xsp, \
             tc.tile_pool(name="psA", bufs=1, space="PSUM") as psA, \
             tc.tile_pool(name="psV", bufs=2, space="PSUM") as psV:
            xT_sb = xsp.tile([128, FCH, T], bf16, name="xT_sb")
            for f in range(FCH):
                nc.gpsimd.dma_start(xT_sb[:, f, :], xT_d[f])
            nc.gpsimd.dma_start(bq_sb[:], bq_d[:])
            nc.gpsimd.dma_start(bk_sb[:], bk_d[:])
            nc.gpsimd.dma_start(kp_sb[:], kpb_d[:])
            nc.gpsimd.dma_start(mq_sb[:], mq_d[:])
            nc.gpsimd.dma_start(mk_sb[:], mk_d[:])
            nc.gpsimd.dma_start(vones_sb[:], vones_d[:])
            wqv_sb = wvp.tile([128, FCH, 512], bf16, name="wqv_sb")
            for f in range(FCH):
                nc.gpsimd.dma_start(wqv_sb[:, f, :], wqkvT_d[f, :, 1024:1536])
            nc.vector.tensor_copy(
                Vt[:, :, :, HD],
                vones_sb[:].rearrange("p (a b) -> p a b", a=NKT))
            for tgt in range(8):
                pqks = [psA.tile([128, 512], f32, name=f"pqk{t5}", tag=f"pqk{t5}")
                        for t5 in range(TQ5)]
                for f in range(FCH):
                    wqt = wqp.tile([128, 128], bf16, name="wqt", tag="wqt")
                    nc.sync.dma_start(
                        wqt[:], wqkvT_d[f, :, tgt * 128:(tgt + 1) * 128])
                    for t5 in range(TQ5):
                        nc.tensor.matmul(
                            pqks[t5][:], wqt[:],
                            xT_sb[:, f, t5 * 512:(t5 + 1) * 512],
                            start=(f == 0), stop=(f == FCH - 1))
                pair = tgt % 4
                for t5 in range(TQ5):
                    dst = (QT if tgt < 4 else KT)[:, pair, t5 * 512:(t5 + 1) * 512]
                    bias = (bq_sb if tgt < 4 else bk_sb)[:, pair:pair + 1]
                    nc.vector.tensor_scalar(dst, pqks[t5][:], bias, None, ADD)
            # V in [t, ch] layout, 128-query subtiles
            for t in range(16):
                pv = psV.tile([128, 512], f32, name="pv", tag="pv0")
                for f in range(FCH):
                    nc.tensor.matmul(
                        pv[:], xT_sb[:, f, t * 128:(t + 1) * 128],
                        wqv_sb[:, f, :],
                        start=(f == 0), stop=(f == FCH - 1))
                nc.vector.tensor_copy(
                    Vt[:, t, :, 0:HD],
                    pv[:].rearrange("p (h d) -> p h d", h=HPG))

        # ---- Phases B+C: banded attention fused with output projection ----
        with tc.tile_pool(name="pers2", bufs=1) as pers2:
            AT = pers2.tile([128, NPAIR, T], bf16, name="AT")

            with tc.tile_pool(name="eps", bufs=4) as epool, \
                 tc.tile_pool(name="nsb", bufs=2) as nsb, \
                 tc.tile_pool(name="avp", bufs=1) as avp, \
                 tc.tile_pool(name="wo", bufs=1) as wop, \
                 tc.tile_pool(name="osb", bufs=2) as osb, \
                 tc.tile_pool(name="psAv", bufs=1, space="PSUM") as psAv, \
                 tc.tile_pool(name="psSc", bufs=2, space="PSUM") as psSc, \
                 tc.tile_pool(name="psC", bufs=1, space="PSUM") as psC:
                wo_sb = wop.tile([128, NPAIR, D], bf16, name="wo_sb")
                for cchunk in range(NPAIR):
                    nc.gpsimd.dma_start(wo_sb[:, cchunk, :], woutT_d[cchunk])

                def mk_normalize(q5, p, avsA, avsB, dpl):
                    # per-pair chain, entirely off the tensor engine: fast
                    # reciprocal of the two gathered denominator rows (both
                    # on partition 0), one gpsimd broadcast per head of its
                    # [1, 512] row to 64 partitions, then two multiplies
                    # writing the bf16 AT slice (head A partitions 0-63,
                    # head B 64-127).
                    def emit():
                        qs = slice(q5 * 512, (q5 + 1) * 512)
                        rf = nsb.tile([1, 2, 512], f32, name="rf", tag="rf")
                        nc.vector.reciprocal_approx_fast(rf[:], dpl[:])
                        bcA = nsb.tile([64, 512], f32, name="bcA", tag="bcA")
                        bcB = nsb.tile([64, 512], f32, name="bcB", tag="bcB")
                        nc.gpsimd.partition_broadcast(bcA[:], rf[0:1, 0, :])
                        nc.gpsimd.partition_broadcast(bcB[:], rf[0:1, 1, :])
                        nc.vector.tensor_tensor(
                            AT[0:64, p, qs], avsA[0:64, :], bcA[:], MUL)
                        nc.vector.tensor_tensor(
                            AT[64:128, p, qs], avsB[0:64, :], bcB[:], MUL)
                    return emit

                def mk_proj_t(q5, tq):
                    # output projection for one 128-query tile of block q5;
                    # emitted as four separate closures so the matmuls
                    # interleave with scores and the scalar engine never
                    # runs dry behind a monolithic projection block.
                    def emit():
                        t = 4 * q5 + tq
                        tsl = slice(t * 128, (t + 1) * 128)
                        po0 = psC.tile([128, 512], f32, name="po0", tag="po0")
                        po1 = psC.tile([128, 512], f32, name="po1", tag="po1")
                        for cchunk in range(NPAIR):
                            lhsT = AT[:, cchunk, tsl]
                            nc.tensor.matmul(
                                po0[:], lhsT, wo_sb[:, cchunk, 0:512],
                                start=(cchunk == 0), stop=(cchunk == 3))
                            nc.tensor.matmul(
                                po1[:], lhsT, wo_sb[:, cchunk, 512:1024],
                                start=(cchunk == 0), stop=(cchunk == 3))
                        ot = osb.tile([128, D], f32, name="ot", tag="ot")
                        nc.vector.tensor_copy(ot[:, 0:512], po0[:])
                        nc.vector.tensor_copy(ot[:, 512:1024], po1[:])
                        nc.sync.dma_start(out_d[tsl, :], ot[:])
                    return emit

                defq = []  # deferred normalize/proj closures, FIFO
                for q5 in range(TQ5):
                    nkt = min(4 * q5 + 5, NKT)
                    q5s = q5 * 512
                    for p in range(NPAIR):
                        avA = psAv.tile([HD + 1, 512], f32, name="avA", tag="avA")
                        avB = psAv.tile([HD + 1, 512], f32, name="avB", tag="avB")
                        # depth-2 AV deferral queue: av(kt) runs after
                        # scores(kt+2) is issued, so exp(kt+1)/exp(kt+2) never
                        # wait behind it on the scalar engine.
                        pend_av = []
                        for kt in range(nkt):
                            ks = slice(kt * 128, (kt + 1) * 128)
                            off = kt - 4 * q5
                            masked = off >= 0
                            # masked tiles only affect queries >= q0
                            q0 = max(0, 128 * off - L) if masked else 0
                            qs = slice(q5s + q0, q5s + 512)
                            sc2 = psSc.tile([128, 2, 512], f32, name="sc2", tag="sc2")
                            nc.tensor.matmul(sc2[:, 0, q0:512],
                                             KT[0:64, p, ks], QT[0:64, p, qs],
                                             start=True, stop=not masked,
                                             tile_position=(0, 0))
                            nc.tensor.matmul(sc2[:, 1, q0:512],
                                             KT[64:128, p, ks], QT[64:128, p, qs],
                                             start=True, stop=not masked,
                                             tile_position=(64, 0))
                            if masked:
                                m1 = min(512, 128 * off + 126)
                                nc.tensor.matmul(sc2[:, 0, q0:m1], mk_sb[:],
                                                 mq_sb[:, off, q0:m1],
                                                 start=False, stop=True,
                                                 skip_group_check=True)
                                nc.tensor.matmul(sc2[:, 1, q0:m1], mk_sb[:],
                                                 mq_sb[:, off, q0:m1],
                                                 start=False, stop=True,
                                                 skip_group_check=True)
                            if len(pend_av) == 2:
                                pend_av.pop(0)()
                            e2 = epool.tile([128, 2, 512], bf16, name="e2", tag="e2")
                            nc.scalar.activation(e2[:, :, q0:512], sc2[:, :, q0:512],
                                                 Exp, bias=kp_sb[:, kt:kt + 1],
                                                 scale=1.0 / math.sqrt(HD))

                            def mk_av(kt=kt, e2=e2, q0=q0, avA=avA, avB=avB,
                                      p=p, nkt=nkt):
                                nc.tensor.matmul(avA[0:65, q0:512],
                                                 Vt[:, kt, 2 * p, :],
                                                 e2[:, 0, q0:512],
                                                 start=(kt == 0), stop=(kt == nkt - 1),
                                                 skip_group_check=True)
                                nc.tensor.matmul(avB[0:65, q0:512],
                                                 Vt[:, kt, 2 * p + 1, :],
                                                 e2[:, 1, q0:512],
                                                 start=(kt == 0), stop=(kt == nkt - 1),
                                                 skip_group_check=True)
                            pend_av.append(mk_av)
                            if kt in (3, nkt - 1) and defq:
                                defq.pop(0)()
                        for fn in pend_av:
                            fn()
                        # copy AV out of PSUM (frees banks) + gather the
                        # denominator rows on the gpsimd DMA queue (so they
                        # don't wait behind weight/output DMAs)
                        avsA = avp.tile([HD + 1, 512], f32,
                                        name=f"avsA{p}", tag=f"avsA{p}")
                        avsB = avp.tile([HD + 1, 512], f32,
                                        name=f"avsB{p}", tag=f"avsB{p}")
                        nc.vector.tensor_copy(avsA[:], avA[:])
                        nc.vector.tensor_copy(avsB[:], avB[:])
                        dpl = nsb.tile([1, 2, 512], f32, name=f"dp{p}", tag=f"dp{p}")
                        nc.gpsimd.dma_start(dpl[0:1, 0, :], avsA[64:65, :])
                        nc.gpsimd.dma_start(dpl[0:1, 1, :], avsB[64:65, :])
                        defq.append(mk_normalize(q5, p, avsA, avsB, dpl))
                    for tq in range(4):
                        defq.append(mk_proj_t(q5, tq))
                # flush the last block's normalize + projection
                for fn in defq:
                    fn()

    nc.finalize()
    return nc


def _host_inputs(x, key_padding_mask, w_qkv, b_qkv, w_out):
    """Per-core input dicts."""
    import ml_dtypes

    f32 = np.float32
    b16 = ml_dtypes.bfloat16
    # masks (shared across cores)
    j = np.arange(128)[:, None]
    q = np.arange(512)[None, :]
    mq = np.zeros((128, 5, 512), f32)
    for off in range(5):
        mq[:, off, :] = (128 * off + j > q + L).astype(f32)
    mq = mq.astype(b16)
    mk = (NEG * np.eye(128, dtype=f32)).astype(b16)
    vones = np.ones((128, NKT * HPG), b16)

    in_maps = []
    for c in range(NCORES):
        b, g = divmod(c, 2)
        # channel rows for this group's Q/K (pairs of heads -> 128 rows each)
        qrows = np.concatenate(
            [w_qkv[64 * (8 * g + 2 * p):64 * (8 * g + 2 * p) + 128] for p in range(NPAIR)])
        krows = np.concatenate(
            [w_qkv[D + 64 * (8 * g + 2 * p):D + 64 * (8 * g + 2 * p) + 128] for p in range(NPAIR)])
        vrows = w_qkv[2 * D + 512 * g:2 * D + 512 * g + 512]
        w_all = np.concatenate([qrows, krows, vrows], 0)          # [1536, 1024]
        wqkvT = np.ascontiguousarray(w_all.T).reshape(FCH, 128, 3 * 512)
        bq = np.stack(
            [b_qkv[64 * (8 * g + 2 * p):64 * (8 * g + 2 * p) + 128] for p in range(NPAIR)], 1)
        bk = np.stack(
            [b_qkv[D + 64 * (8 * g + 2 * p):D + 64 * (8 * g + 2 * p) + 128] for p in range(NPAIR)], 1)
        xT = np.ascontiguousarray(x[b].T).reshape(FCH, 128, T)
        woutT = np.ascontiguousarray(w_out.T[512 * g:512 * g + 512]).reshape(NPAIR, 128, D)
        kpb = np.ascontiguousarray(
            (NEG * key_padding_mask[b].astype(f32)).reshape(NKT, 128).T)
        in_maps.append({
            "xT": xT.astype(b16), "wqkvT": wqkvT.astype(b16),
            "woutT": woutT.astype(b16),
            "bq": bq.astype(f32), "bk": bk.astype(f32), "kpb": kpb.astype(f32),
            "mq": mq, "mk": mk, "vones": vones,
        })
    return in_maps


def kernel(x, key_padding_mask, w_qkv, b_qkv, w_out, b_out):
    from concourse.bass_utils import run_bass_kernel_spmd

    x = np.asarray(x, np.float32)
    key_padding_mask = np.asarray(key_padding_mask)
    w_qkv = np.asarray(w_qkv, np.float32)
    b_qkv = np.asarray(b_qkv, np.float32)
    w_out = np.asarray(w_out, np.float32)
    b_out = np.asarray(b_out, np.float32)

    if "nc" not in _BUILT:
        _BUILT["nc"] = _build_nc()
    nc = _BUILT["nc"]

    in_maps = _host_inputs(x, key_padding_mask, w_qkv, b_qkv, w_out)
    res = run_bass_kernel_spmd(nc, in_maps, core_ids=list(range(NCORES)))
    out = np.empty((B, T, D), np.float32)
    for b in range(B):
        out[b] = res.results[2 * b]["out_part"] + res.results[2 * b + 1]["out_part"]
    # host-folded biases: b_out plus the V-bias pushed through the projection
    bv = b_qkv[2 * D:3 * D]
    out += (b_out + bv @ w_out.T)[None, None, :].astype(np.float32)
    return out
